# revision 23
# baseline (speedup 1.0000x reference)
"""Trainium2 Bass kernel for nn_CausalVAE2 (encoder + per-node masked decoder).

Computation (per sample, d=64, hid=256):
  h   = relu(x @ enc1_w + enc1_b)                 # [256]
  z   = h @ enc3_w + enc3_b                       # [128] -> mu, log_var
  ey  = mu + exp(0.5*log_var) * eps               # [64]
  per node i: x_out[i] = relu(ey @ W1m[i] + dec1_b[i]) @ dec3_w[i] + dec3_b[i]
     where W1m[i] = dec1_w[i] * mask_in[i][:, None]
  y_recon = [x_out, zeros]

Sharding: pure data parallelism over the batch axis across 8 NeuronCores.

On-chip layout ("option C"): batch on SBUF/PSUM partitions (tiles of 128
samples), features on the free axis.  Biases are folded into the matmuls via
an extra ones-row on the stationary operand (K=65).  The decoder drain
(relu -> *dec3_w -> row-sum) is a single fused DVE scalar_tensor_tensor per
node reading PSUM directly.
"""

import numpy as np

import concourse.bass as bass
import concourse.mybir as mybir
import concourse.tile as tile
from concourse.bass_utils import run_bass_kernel_spmd
from concourse.vector_clock import ScopedClock


class TCSafe(tile.TileContext):
    """TileContext that splits per-instruction semaphore waits to respect
    walrus's per-opcode sync-wait slot limit (1 wait per instruction on this
    compiler build): excess waits are hoisted onto injected same-engine NOPs
    immediately before the instruction."""

    _WAIT_LIMITS: dict = {}

    def _drain_and_barrier(self, tick_clock, wait_clock):
        super()._drain_and_barrier(tick_clock, wait_clock)
        self._split_excess_waits()

    def _split_excess_waits(self):
        nc = self.nc
        for bb in nc.main_func.blocks:
            insts = bb.instructions
            i = 0
            while i < len(insts):
                inst = insts[i]
                si = inst.sync_info
                waits = list(si.on_wait) if si is not None and si.on_wait else []
                limit = self._WAIT_LIMITS.get(type(inst).__name__, 1)
                if len(waits) > limit and inst.engine != mybir.EngineType.Unassigned:
                    keep = waits[-limit:]
                    extra = waits[:-limit]
                    si.on_wait = keep
                    for k, w in enumerate(extra):
                        nop = mybir.InstNoOp(
                            name=nc.get_next_instruction_name(),
                            ins=[], outs=[],
                            engine=inst.engine,
                            sync_info=mybir.SyncInfo(on_wait=[w], on_update=[]),
                        )
                        nc.register_instruction(nop, overwrite=True)
                        insts.insert(i + k, nop)
                    i += len(extra)
                i += 1

D = 64
HID = 256
NODES = 64
TWO_D = 2 * D
TILE_P = 128           # samples per tile (partition dim)
FULL_BATCH = 262144
N_CORES = 8

f32 = mybir.dt.float32
f32r = mybir.dt.float32r
bf16 = mybir.dt.bfloat16
Alu = mybir.AluOpType
Act = mybir.ActivationFunctionType

# ---- decoder drain engine assignment (by PSUM-bank pair of 2 nodes) ----
# 'dve': fused relu*w3+rowsum on VectorE straight from PSUM
# 'act': |w3| sign-folded into the matmul; 2 relu+accum ops on ScalarE
# 'gp' : ScalarE relu -> bf16 SBUF, then fused mul+rowsum on GpSimd
GP_PAIRS = 0
ACT_PAIRS = 0


def _pair_classes():
    cls = []
    for p in range(NODES // 2):
        if p < GP_PAIRS:
            cls.append("gp")
        elif p < GP_PAIRS + ACT_PAIRS:
            cls.append("act")
        else:
            cls.append("dve")
    return cls


def build_nc(n_tiles: int, classes=None, cps=None) -> bass.Bass:
    """Build the single-core Bass program for a per-core batch of
    n_tiles * 128 samples (SPMD: same program on every core)."""
    if classes is None:
        classes = _pair_classes()
    if cps is None:
        cps = [HID // 2] * NODES
    use_gp = any(c == "gp" for c in classes)
    use_act = any(c == "act" for c in classes)
    bc = n_tiles * TILE_P
    nc = bass.Bass()

    # ---- DRAM I/O ----
    x_d = nc.dram_tensor("x", [bc, D + 1], f32, kind="ExternalInput")
    eps_d = nc.dram_tensor("eps", [bc, D], f32, kind="ExternalInput")
    # host-prepped weights
    w1_d = nc.dram_tensor("w1aug", [D + 1, NODES * HID], f32r, kind="ExternalInput")
    w3_d = nc.dram_tensor("w3rep", [TILE_P, NODES * HID], f32, kind="ExternalInput")
    w3b_d = nc.dram_tensor("w3bf", [TILE_P, NODES * HID], bf16, kind="ExternalInput")
    b3_d = nc.dram_tensor("b3rep", [TILE_P, NODES], f32, kind="ExternalInput")
    e1w_d = nc.dram_tensor("e1waug", [D + 1, HID], f32, kind="ExternalInput")
    e3w_d = nc.dram_tensor("e3wpk", [TILE_P, TWO_D * 2], f32, kind="ExternalInput")
    e3b_d = nc.dram_tensor("e3b", [1, TWO_D], f32, kind="ExternalInput")
    id_d = nc.dram_tensor("ident", [TILE_P, TILE_P], f32, kind="ExternalInput")

    xo_d = nc.dram_tensor("x_out", [bc, D], f32, kind="ExternalOutput")
    mu_d = nc.dram_tensor("mu", [bc, D], f32, kind="ExternalOutput")
    lv_d = nc.dram_tensor("log_var", [bc, D], f32, kind="ExternalOutput")
    yr_d = nc.dram_tensor("y_recon", [bc, TWO_D], f32, kind="ExternalOutput")

    with TCSafe(nc) as tc:
        with (
            tc.tile_pool(name="const", bufs=1) as cpool,
            tc.tile_pool(name="work", bufs=3) as wpool,
            tc.tile_pool(name="scr", bufs=4) as spool,
            tc.tile_pool(name="ps_enc", bufs=3, space=bass.MemorySpace.PSUM) as ps_enc,
            tc.tile_pool(name="ps_z", bufs=1, space=bass.MemorySpace.PSUM) as ps_z,
            tc.tile_pool(name="ps_dec", bufs=4, space=bass.MemorySpace.PSUM) as ps_dec,
        ):
            # ---- constants into SBUF ----
            w1_sb = cpool.tile([D + 1, NODES * HID], f32r, tag="w1")
            nc.sync.dma_start(w1_sb[:], w1_d[:])
            w3_sb = cpool.tile([TILE_P, NODES * HID], f32, tag="w3")
            nc.sync.dma_start(w3_sb[:], w3_d[:])
            if use_gp:
                w3b_sb = cpool.tile([TILE_P, NODES * HID], bf16, tag="w3b")
                nc.sync.dma_start(w3b_sb[:], w3b_d[:])
            b3_sb = cpool.tile([TILE_P, NODES], f32, tag="b3")
            nc.sync.dma_start(b3_sb[:], b3_d[:])
            e1w_sb = cpool.tile([D + 1, HID], f32, tag="e1w")
            nc.sync.dma_start(e1w_sb[:], e1w_d[:])
            e3w_sb = cpool.tile([TILE_P, TWO_D * 2], f32, tag="e3w")
            nc.sync.dma_start(e3w_sb[:], e3w_d[:])
            e3b_sb = cpool.tile([1, TWO_D], f32, tag="e3b")
            nc.sync.dma_start(e3b_sb[:], e3b_d[:])
            ident_sb = cpool.tile([TILE_P, TILE_P], f32, tag="ident")
            nc.sync.dma_start(ident_sb[:], id_d[:])
            ones_sb = cpool.tile([1, TILE_P], f32, tag="ones")
            nc.gpsimd.memset(ones_sb[:], 1.0)
            zeros_sb = cpool.tile([TILE_P, D], f32, tag="zeros")
            nc.gpsimd.memset(zeros_sb[:], 0.0)

            for t in range(n_tiles):
                r0, r1 = t * TILE_P, (t + 1) * TILE_P
                # ---- load inputs ----
                x_t = wpool.tile([TILE_P, D + 1], f32, tag="x_t")
                nc.sync.dma_start(x_t[:], x_d[r0:r1, :])
                eps_t = wpool.tile([TILE_P, D], f32, tag="eps_t")
                nc.sync.dma_start(eps_t[:], eps_d[r0:r1, :])

                # ---- encoder ----
                # xT: [128, 65] -> [65, 128] via PE transpose
                xT_ps = ps_enc.tile([D + 1, TILE_P], f32, tag="encps")
                nc.tensor.transpose(xT_ps[:], x_t[:], ident_sb[:])
                xT_sb = wpool.tile([D + 1, TILE_P], f32, tag="xT_sb")
                nc.scalar.copy(xT_sb[:], xT_ps[:])

                # enc layer1: hT chunks [128 hid, 128 batch]
                hT_sb = wpool.tile([TILE_P, 2 * TILE_P], f32, tag="hT_sb")
                for c in range(2):
                    hT_ps = ps_enc.tile([TILE_P, TILE_P], f32, tag="encps")
                    nc.tensor.matmul(
                        hT_ps[:],
                        lhsT=e1w_sb[:, c * TILE_P : (c + 1) * TILE_P],
                        rhs=xT_sb[:],
                    )
                    nc.scalar.activation(
                        hT_sb[:, c * TILE_P : (c + 1) * TILE_P], hT_ps[:], Act.Relu
                    )

                # enc layer2: z [128 batch, 128]
                z_ps = ps_z.tile([TILE_P, TWO_D], f32, tag="z")
                nc.tensor.matmul(
                    z_ps[:], lhsT=hT_sb[:, 0:TILE_P], rhs=e3w_sb[:, 0:TWO_D],
                    start=True, stop=False,
                )
                nc.tensor.matmul(
                    z_ps[:], lhsT=hT_sb[:, TILE_P:], rhs=e3w_sb[:, TWO_D:],
                    start=False, stop=False,
                )
                nc.tensor.matmul(
                    z_ps[:], lhsT=ones_sb[:], rhs=e3b_sb[:],
                    start=False, stop=True,
                )

                mu_sb = wpool.tile([TILE_P, D], f32, tag="mu_sb")
                nc.scalar.copy(mu_sb[:], z_ps[:, 0:D])
                lv_sb = wpool.tile([TILE_P, D], f32, tag="lv_sb")
                nc.scalar.copy(lv_sb[:], z_ps[:, D:TWO_D])
                s_sb = wpool.tile([TILE_P, D], f32, tag="s_sb")
                nc.scalar.activation(s_sb[:], z_ps[:, D:TWO_D], Act.Exp,
                                     bias=0.0, scale=0.5)
                t_sb = wpool.tile([TILE_P, D], f32, tag="t_sb")
                nc.vector.tensor_tensor(t_sb[:], s_sb[:], eps_t[:], Alu.mult)
                ey = wpool.tile([TILE_P, D + 1], f32, tag="ey")
                nc.vector.tensor_tensor(ey[:, 0:D], t_sb[:], mu_sb[:], Alu.add)
                nc.gpsimd.memset(ey[:, D : D + 1], 1.0)

                # eyT: [65, 128] stationary for the decoder
                eT_ps = ps_enc.tile([D + 1, TILE_P], f32, tag="encps")
                nc.tensor.transpose(eT_ps[:], ey[:], ident_sb[:])
                eT_sb = wpool.tile([D + 1, TILE_P], f32r, tag="eT_sb")
                nc.scalar.copy(eT_sb[:], eT_ps[:])

                # ---- decoder: 64 nodes, 2 per PSUM bank ----
                xacc = wpool.tile([TILE_P, NODES], f32, tag="xacc")
                if use_act:
                    xaccN = wpool.tile([TILE_P, NODES], f32, tag="xaccN")
                    nc.gpsimd.memset(xaccN[:], 0.0)
                for pair in range(NODES // 2):
                    kind = classes[pair]
                    bank = ps_dec.tile([TILE_P, 2 * HID], f32, tag="dec")
                    for h in range(2):
                        i = 2 * pair + h
                        nc.tensor.matmul(
                            bank[:, h * HID : (h + 1) * HID],
                            lhsT=eT_sb[:],
                            rhs=w1_sb[:, i * HID : (i + 1) * HID],
                        )
                    if kind == "dve":
                        for h in range(2):
                            i = 2 * pair + h
                            scr = spool.tile([TILE_P, HID], bf16, tag="scr")
                            # out = max(psum, 0) * w3 ; accum = row-sum(out)
                            nc.vector.scalar_tensor_tensor(
                                out=scr[:],
                                in0=bank[:, h * HID : (h + 1) * HID],
                                scalar=0.0,
                                in1=w3_sb[:, i * HID : (i + 1) * HID],
                                op0=Alu.max,
                                op1=Alu.mult,
                                accum_out=xacc[:, i : i + 1],
                            )
                    elif kind == "act":
                        # |w3| folded into the matmul; columns reordered so
                        # positive-w3 columns come first (cps[i] of them).
                        for h in range(2):
                            i = 2 * pair + h
                            cp = cps[i]
                            scrP = spool.tile([TILE_P, HID], bf16, tag="scrA")
                            nc.scalar.activation(
                                scrP[:, 0:cp],
                                bank[:, h * HID : h * HID + cp],
                                Act.Relu,
                                accum_out=xacc[:, i : i + 1],
                            )
                            nc.scalar.activation(
                                scrP[:, cp:HID],
                                bank[:, h * HID + cp : (h + 1) * HID],
                                Act.Relu,
                                accum_out=xaccN[:, i : i + 1],
                            )
                    else:  # gp
                        relu_sb = spool.tile([TILE_P, 2 * HID], bf16, tag="relu")
                        nc.scalar.activation(relu_sb[:], bank[:], Act.Relu)
                        for h in range(2):
                            i = 2 * pair + h
                            gscr = spool.tile([TILE_P, HID], bf16, tag="gscr")
                            nc.gpsimd.scalar_tensor_tensor(
                                out=gscr[:],
                                in0=relu_sb[:, h * HID : (h + 1) * HID],
                                scalar=0.0,
                                in1=w3b_sb[:, i * HID : (i + 1) * HID],
                                op0=Alu.max,
                                op1=Alu.mult,
                                accum_out=xacc[:, i : i + 1],
                            )

                xo = wpool.tile([TILE_P, NODES], f32, tag="xo")
                if use_act:
                    # xo = (xacc - xaccN) + b3
                    xo1 = wpool.tile([TILE_P, NODES], f32, tag="xo1")
                    nc.vector.scalar_tensor_tensor(
                        out=xo1[:], in0=xaccN[:], scalar=-1.0, in1=xacc[:],
                        op0=Alu.mult, op1=Alu.add,
                    )
                    nc.vector.tensor_tensor(xo[:], xo1[:], b3_sb[:], Alu.add)
                else:
                    nc.vector.tensor_tensor(xo[:], xacc[:], b3_sb[:], Alu.add)

                # ---- stores ----
                nc.gpsimd.dma_start(xo_d[r0:r1, :], xo[:])
                nc.gpsimd.dma_start(yr_d[r0:r1, 0:D], xo[:])
                nc.gpsimd.dma_start(mu_d[r0:r1, :], mu_sb[:])
                nc.gpsimd.dma_start(lv_d[r0:r1, :], lv_sb[:])
                nc.gpsimd.dma_start(yr_d[r0:r1, D:TWO_D], zeros_sb[:])
    return nc


def prep_weights(enc1_w, enc1_b, enc3_w, enc3_b,
                 dec1_w, dec1_b, dec3_w, dec3_b, B_mask):
    """Host-side packing of the tiny weights (all fp32, exact)."""
    enc1_w = np.asarray(enc1_w, np.float32)
    enc1_b = np.asarray(enc1_b, np.float32)
    enc3_w = np.asarray(enc3_w, np.float32)
    enc3_b = np.asarray(enc3_b, np.float32)
    dec1_w = np.asarray(dec1_w, np.float32)
    dec1_b = np.asarray(dec1_b, np.float32)
    dec3_w = np.asarray(dec3_w, np.float32)
    dec3_b = np.asarray(dec3_b, np.float32)
    mask_in = np.asarray(B_mask).T.astype(np.float32)      # [node, in]

    w1m = dec1_w * mask_in[:, :, None]                     # [node, in, hid]
    classes = _pair_classes()
    # per node block [65, 256] = vstack(W1m[i], dec1_b[i]) ; concat along free.
    # For 'act' nodes, |dec3_w| is folded into the columns, which are permuted
    # so positive-dec3_w columns come first (cps[i] of them).
    w1aug = np.empty((D + 1, NODES * HID), np.float32)
    w3eff = dec3_w.copy()
    cps = [0] * NODES
    for i in range(NODES):
        blk_w = w1m[i]
        blk_b = dec1_b[i]
        if classes[i // 2] == "act":
            pos = dec3_w[i] > 0
            perm = np.argsort(~pos, kind="stable")
            f = np.abs(dec3_w[i])[perm]
            blk_w = blk_w[:, perm] * f
            blk_b = blk_b[perm] * f
            cps[i] = int(pos.sum())
        w1aug[0:D, i * HID : (i + 1) * HID] = blk_w
        w1aug[D, i * HID : (i + 1) * HID] = blk_b
    w3rep = np.tile(w3eff.reshape(1, NODES * HID), (TILE_P, 1)).astype(np.float32)
    b3rep = np.tile(dec3_b.reshape(1, NODES), (TILE_P, 1)).astype(np.float32)
    e1waug = np.concatenate([enc1_w, enc1_b.reshape(1, HID)], axis=0)
    e3wpk = np.concatenate([enc3_w[0:TILE_P, :], enc3_w[TILE_P:, :]], axis=1)
    e3b = enc3_b.reshape(1, TWO_D)
    ident = np.eye(TILE_P, dtype=np.float32)
    try:
        import ml_dtypes
        w3bf = w3rep.astype(ml_dtypes.bfloat16)
    except ImportError:
        w3bf = w3rep.astype(np.float32).view(np.uint32)
        w3bf = ((w3bf + 0x8000) >> 16).astype(np.uint16)  # rne-ish to bf16 bits
    consts = dict(w1aug=w1aug, w3rep=w3rep, w3bf=w3bf, b3rep=b3rep,
                  e1waug=e1waug, e3wpk=np.ascontiguousarray(e3wpk), e3b=e3b,
                  ident=ident)
    return consts, cps


def kernel(x, eps, enc1_w, enc1_b, enc3_w, enc3_b,
           dec1_w, dec1_b, dec3_w, dec3_b, B_mask, _trace=False):
    x = np.asarray(x, np.float32)
    eps = np.asarray(eps, np.float32)
    batch = x.shape[0]
    assert batch % (N_CORES * TILE_P) == 0
    bc = batch // N_CORES
    n_tiles = bc // TILE_P

    consts, cps = prep_weights(enc1_w, enc1_b, enc3_w, enc3_b,
                               dec1_w, dec1_b, dec3_w, dec3_b, B_mask)

    nc = build_nc(n_tiles, cps=cps)
    in_maps = []
    for c in range(N_CORES):
        m = dict(consts)
        xs = x[c * bc : (c + 1) * bc]
        m["x"] = np.concatenate(
            [xs, np.ones((bc, 1), np.float32)], axis=1)
        m["eps"] = np.ascontiguousarray(eps[c * bc : (c + 1) * bc])
        in_maps.append(m)

    res = run_bass_kernel_spmd(nc, in_maps, list(range(N_CORES)), trace=False)
    outs = res.results
    x_out = np.concatenate([outs[c]["x_out"] for c in range(N_CORES)], axis=0)
    mu = np.concatenate([outs[c]["mu"] for c in range(N_CORES)], axis=0)
    log_var = np.concatenate([outs[c]["log_var"] for c in range(N_CORES)], axis=0)
    y_recon = np.concatenate([outs[c]["y_recon"] for c in range(N_CORES)], axis=0)
    kernel._last_result = res
    if _trace:
        kernel._last_times = timed_run(nc, in_maps)
    return (x_out, mu, log_var, y_recon)


def timed_run(nc, in_maps, iters=12):
    """Re-run the compiled program on 8 cores with device-resident inputs and
    wall-clock each dispatch (upper bound on HW exec time)."""
    import time
    import jax
    from jax.sharding import Mesh, PartitionSpec
    from jax.experimental.shard_map import shard_map
    import concourse.mybir as mybir_
    from concourse import bass2jax
    from concourse.bass2jax import _bass_exec_p

    bass2jax.install_neuronx_cc_hook()
    n_cores = len(in_maps)
    part_name = nc.partition_id_tensor.name if nc.partition_id_tensor else None
    in_names, out_names, out_avals = [], [], []
    for alloc in nc.m.functions[0].allocations:
        if not isinstance(alloc, mybir_.MemoryLocationSet):
            continue
        name = alloc.memorylocations[0].name
        if alloc.kind == "ExternalInput":
            if name != part_name:
                in_names.append(name)
        elif alloc.kind == "ExternalOutput":
            out_names.append(name)
            out_avals.append(jax.core.ShapedArray(
                tuple(alloc.tensor_shape), mybir_.dt.np(alloc.dtype)))
    n_params = len(in_names)
    all_names = in_names + out_names
    if part_name is not None:
        all_names = all_names + [part_name]

    def _body(*args):
        operands = list(args)
        if part_name is not None:
            operands.append(bass2jax.partition_id_tensor())
        return tuple(_bass_exec_p.bind(
            *operands, out_avals=tuple(out_avals), in_names=tuple(all_names),
            out_names=tuple(out_names), lowering_input_output_aliases=(),
            sim_require_finite=True, sim_require_nnan=True, nc=nc))

    devices = jax.devices()[:n_cores]
    mesh = Mesh(np.asarray(devices), ("core",))
    n_outs = len(out_names)
    fn = jax.jit(shard_map(_body, mesh=mesh,
                           in_specs=(PartitionSpec("core"),) * (n_params + n_outs),
                           out_specs=(PartitionSpec("core"),) * n_outs,
                           check_rep=False), keep_unused=True)
    concat_in = [np.concatenate([np.asarray(in_maps[c][nm])
                                 for c in range(n_cores)], axis=0)
                 for nm in in_names]
    concat_zero = [np.zeros((n_cores * a.shape[0], *a.shape[1:]), a.dtype)
                   for a in out_avals]
    args = [jax.device_put(a) for a in concat_in + concat_zero]
    out = fn(*args)
    jax.block_until_ready(out)        # warm-up / compile
    times = []
    for _ in range(iters):
        t0 = time.perf_counter()
        out = fn(*args)
        jax.block_until_ready(out)
        times.append(time.perf_counter() - t0)
    return times
